# revision 16
# baseline (speedup 1.0000x reference)
"""Causal self-attention (B=4, T=2048, C=1024, H=16, D=64) on 8 trn2 NeuronCores.

Sharding: core c = (batch b = c//2, head-group g = c%2). Megatron-style within a
batch: each core computes 8 heads' q/k/v (column-parallel) and a row-parallel
partial out-projection. Host sums the two partials per batch and adds the
rank-1 bias term (bo + bv @ wo) -- valid because softmax rows sum to 1, so v's
bias never needs to enter the kernel.

Per-core kernel (all matmuls bf16, fp32 PSUM accumulation):
  phase 1 (per 512-wide T chunk): qT,kT = (x@w)^T via lhsT=w, rhs=x^T (host
           pre-transposes x); v natural via lhsT=x^T-chunk, rhs=wv; a ones
           column is appended to each head's v block.
  phase 2: flash-style streaming attention in S^T orientation:
           S^T[k,q] = kT.T @ qT (head pairs packed in PE row groups 0/64),
           P^T = exp(S^T) (ScalarE, 1/sqrt(D) folded into q), causal masking
           by mask-multiply (GpSimd) on diagonal tiles only; O^T accumulated
           via lhsT=v_tile (stationary), rhs=P^T; the ones column of v makes
           PSUM row 64 the softmax denominator Z for free. 1/Z = Exp(-Log(Z))
           on ScalarE (same activation table set as Exp), broadcast across
           partitions with a K=1 ones outer-product on the PE.
  phase 3 (per T chunk, overlapped with the next chunk's attention):
           y = O @ wo via lhsT=O^T (already the natural layout), rhs=wo.
"""
import numpy as np
import ml_dtypes

import concourse.tile as tile
from concourse import bacc, mybir
from concourse.bass_utils import run_bass_kernel_spmd

BF16 = ml_dtypes.bfloat16
F32 = mybir.dt.float32
BT16 = mybir.dt.bfloat16
AF = mybir.ActivationFunctionType
ALU = mybir.AluOpType

B, T, C, H, D = 4, 2048, 1024, 16, 64
G = 2              # head groups (cores per batch)
HL = H // G        # heads per core = 8
HD = HL * D        # local head dims = 512
NP = 4             # head pairs per core
NJQ = T // 512     # q chunks of 512 = 4
NIK = T // 128     # k tiles of 128 = 16
KC = C // 128      # contraction chunks = 8

_CACHED = {}


def _build():
    nc = bacc.Bacc("TRN2", debug=False)
    xT = nc.dram_tensor("xT", [C, T], BT16, kind="ExternalInput").ap()
    wq = nc.dram_tensor("wq", [C, HD], BT16, kind="ExternalInput").ap()
    wk = nc.dram_tensor("wk", [C, HD], BT16, kind="ExternalInput").ap()
    wv = nc.dram_tensor("wv", [C, HD], BT16, kind="ExternalInput").ap()
    wo = nc.dram_tensor("wo", [HD, C], BT16, kind="ExternalInput").ap()
    bq = nc.dram_tensor("bq", [128, NP], F32, kind="ExternalInput").ap()
    bk = nc.dram_tensor("bk", [128, NP], F32, kind="ExternalInput").ap()
    masks = nc.dram_tensor("masks", [128, 4, 512], BT16, kind="ExternalInput").ap()
    rcp_dram = nc.dram_tensor("rcp_dram", [NJQ, 8, 512], BT16).ap()
    y = nc.dram_tensor("y", [T, C], F32, kind="ExternalOutput").ap()

    with tile.TileContext(nc) as tc:
        with (
            tc.tile_pool(name="consts", bufs=1) as consts,
            tc.tile_pool(name="xt", bufs=3) as xtp,
            tc.tile_pool(name="qk", bufs=1) as qkp,
            tc.tile_pool(name="vp", bufs=1) as vp,
            tc.tile_pool(name="otp", bufs=1) as otp,
            tc.tile_pool(name="pt", bufs=6) as ptp,
            tc.tile_pool(name="ptmp", bufs=3) as ptmpp,
            tc.tile_pool(name="zn", bufs=3) as znp,
            tc.tile_pool(name="yst", bufs=4) as ystp,
            tc.tile_pool(name="ps", bufs=2, space="PSUM") as ps,
        ):
            # ---- constants (wq + first x chunk first: they gate the first matmul) ----
            wq_sb = consts.tile([128, KC, HD], BT16, tag="wq")
            wq_r = wq.rearrange("(k p) c -> p k c", p=128)
            xt0 = xtp.tile([128, KC, 512], BT16, tag="xt", name="xt_pre0")
            x0_r = xT[:, 0:512].rearrange("(k p) t -> p k t", p=128)
            for k in range(KC):
                nc.scalar.dma_start(wq_sb[:, k, :], wq_r[:, k, :])
                nc.sync.dma_start(xt0[:, k, :], x0_r[:, k, :])
            wk_sb = consts.tile([128, KC, HD], BT16, tag="wk")
            wk_r = wk.rearrange("(k p) c -> p k c", p=128)
            for k in range(KC):
                nc.sync.dma_start(wk_sb[:, k, :], wk_r[:, k, :])
            masks_dma = consts.tile([128, 4, 512], BT16, tag="masks_dma")
            masks_sb = consts.tile([128, 4, 512], BT16, tag="masks")
            nc.gpsimd.dma_start(masks_dma, masks)
            nc.gpsimd.tensor_copy(masks_sb, masks_dma)
            wv_sb = consts.tile([128, KC, HD], BT16, tag="wv")
            nc.gpsimd.dma_start(wv_sb, wv.rearrange("(k p) c -> p k c", p=128))
            wo_sb = consts.tile([128, NP, C], BT16, tag="wo")
            nc.gpsimd.dma_start(wo_sb, wo.rearrange("(t p) c -> p t c", p=128))
            bq_dma = consts.tile([128, NP], F32, tag="bq_dma")
            bq_sb = consts.tile([128, NP], F32, tag="bq")
            nc.sync.dma_start(bq_dma, bq)
            nc.vector.tensor_copy(bq_sb, bq_dma)
            bk_dma = consts.tile([128, NP], F32, tag="bk_dma")
            bk_sb = consts.tile([128, NP], F32, tag="bk")
            nc.sync.dma_start(bk_dma, bk)
            nc.vector.tensor_copy(bk_sb, bk_dma)

            # ---- persistent activations ----
            qT = [qkp.tile([128, T], BT16, tag=f"qT{t}", name=f"qT{t}") for t in range(NP)]
            kT = [qkp.tile([128, T], BT16, tag=f"kT{t}", name=f"kT{t}") for t in range(NP)]
            v_sb = [vp.tile([128, HL * 65], BT16, tag=f"v{i}", name=f"v{i}") for i in range(NIK)]
            oT = [otp.tile([128, T], BT16, tag=f"oT{t}", name=f"oT{t}") for t in range(NP)]

            def phase1(jt):
                if jt == 0:
                    xt = xt0
                else:
                    xt = xtp.tile([128, KC, 512], BT16, tag="xt", name=f"xt{jt}")
                    xr = xT[:, jt * 512:(jt + 1) * 512].rearrange("(k p) t -> p k t", p=128)
                    for k in range(KC):
                        (nc.sync if k % 2 == 0 else nc.scalar).dma_start(xt[:, k, :], xr[:, k, :])
                for t in range(NP):
                    p = ps.tile([128, 512], F32, tag="st", name=f"pq{jt}_{t}")
                    for k in range(KC):
                        nc.tensor.matmul(
                            p, wq_sb[:, k, t * 128:(t + 1) * 128], xt[:, k, :],
                            start=(k == 0), stop=(k == KC - 1),
                        )
                    nc.vector.tensor_scalar(
                        qT[t][:, jt * 512:(jt + 1) * 512], p,
                        0.125, bq_sb[:, t:t + 1], ALU.mult, ALU.add,
                    )
                for t in range(NP):
                    p = ps.tile([128, 512], F32, tag="st", name=f"pk{jt}_{t}")
                    for k in range(KC):
                        nc.tensor.matmul(
                            p, wk_sb[:, k, t * 128:(t + 1) * 128], xt[:, k, :],
                            start=(k == 0), stop=(k == KC - 1),
                        )
                    nc.vector.tensor_scalar_add(
                        kT[t][:, jt * 512:(jt + 1) * 512], p, bk_sb[:, t:t + 1]
                    )
                for s in range(4):
                    ik = jt * 4 + s
                    p = ps.tile([128, 512], F32, tag="st", name=f"pv{ik}")
                    for k in range(KC):
                        nc.tensor.matmul(
                            p, xt[:, k, s * 128:(s + 1) * 128], wv_sb[:, k, :],
                            start=(k == 0), stop=(k == KC - 1),
                        )
                    vg = v_sb[ik].rearrange("p (h c) -> p h c", c=65)
                    nc.vector.tensor_copy(
                        vg[:, :, 0:64], p.rearrange("p (h c) -> p h c", c=64)
                    )
                    nc.vector.memset(vg[:, :, 64:65], 1.0)

            def av(t, ik, nik, pts, o_ps):
                pt, c0 = pts[ik]
                ptg = pt.rearrange("p (h q) -> p h q", q=512)
                for hh in range(2):
                    h = 2 * t + hh
                    nc.tensor.matmul(
                        o_ps[hh][:, c0:512], v_sb[ik][:, h * 65:h * 65 + 65],
                        ptg[:, hh, c0:512],
                        start=(ik == 0), stop=(ik == nik - 1),
                    )

            def attention(t, jq):
                nik = 4 * jq + 4
                qs = slice(jq * 512, (jq + 1) * 512)
                o_ps = [
                    ps.tile([65, 512], F32, tag="ot", bufs=4, name=f"ops{t}_{jq}_{_h}")
                    for _h in range(2)
                ]
                pts = {}
                for ik in range(nik):
                    d = ik - 4 * jq
                    c0 = 128 * d if d > 0 else 0   # first potentially-valid column
                    st = ps.tile([128, 1024], F32, tag="st", name=f"st{t}_{jq}_{ik}")
                    stg = st.rearrange("p (h q) -> p h q", q=512)
                    for hh in range(2):
                        r = slice(hh * 64, hh * 64 + 64)
                        nc.tensor.matmul(
                            stg[:, hh, c0:512],
                            kT[t][r, ik * 128:(ik + 1) * 128],
                            qT[t][r, jq * 512 + c0:(jq + 1) * 512],
                            start=True, stop=True,
                        )
                    pt = ptp.tile([128, 1024], BT16, tag="pt", name=f"pt{t}_{jq}_{ik}")
                    ptg = pt.rearrange("p (h q) -> p h q", q=512)
                    if d >= 0:
                        ptm = ptmpp.tile([128, 1024], BT16, tag="ptmp", name=f"ptm{t}_{jq}_{ik}")
                        ptmg = ptm.rearrange("p (h q) -> p h q", q=512)
                        nc.scalar.activation(ptmg[:, :, c0:512], stg[:, :, c0:512], AF.Exp)
                        for hh in range(2):
                            nc.vector.tensor_mul(
                                ptg[:, hh, c0:512],
                                ptmg[:, hh, c0:512],
                                masks_sb[:, d, c0:512],
                            )
                    else:
                        nc.scalar.activation(pt, st, AF.Exp)
                    pts[ik] = (pt, c0)
                    if ik > 0:
                        av(t, ik - 1, nik, pts, o_ps)
                av(t, nik - 1, nik, pts, o_ps)
                # evict Z row + unnormalized O^T, freeing the PSUM accumulators
                out_h = []
                for hh in range(2):
                    ouz = znp.tile([65, 512], F32, tag="ouz", bufs=6, name=f"oz{t}_{jq}_{hh}")
                    nc.vector.tensor_copy(ouz, o_ps[hh])
                    out_h.append(ouz)
                return out_h

            def phase3_m(m):
                    for n in range(2):
                        p = ps.tile([128, 512], F32, tag="st", name=f"py{m}_{n}")
                        for t in range(NP):
                            nc.tensor.matmul(
                                p, oT[t][:, m * 128:(m + 1) * 128],
                                wo_sb[:, t, n * 512:(n + 1) * 512],
                                start=(t == 0), stop=(t == NP - 1),
                            )
                        ys = ystp.tile([128, 512], F32, tag="y", name=f"ys{m}_{n}")
                        nc.vector.tensor_copy(ys, p)
                        nc.gpsimd.dma_start(
                            y[m * 128:(m + 1) * 128, n * 512:(n + 1) * 512], ys
                        )

            def phase3(jq):
                for m in range(4 * jq, 4 * jq + 4):
                    phase3_m(m)

            import concourse.bass as bass_mod

            def normalize(t, jq, evicted, interleave_phase3=False):
                # evicted: [(ouz_h0, ...), (ouz_h1, ...)] for pair t at chunk jq.
                # Pack both heads' Z rows [1,512] as [8,64] each -> one [16,64]
                # reciprocal (64 elems/lane), then broadcast 1/Z via a DRAM
                # round-trip (partition-step-0 DMA reads are legal from DRAM).
                qs2 = slice(jq * 512, (jq + 1) * 512)
                zb = znp.tile([16, 64], F32, tag="zb", bufs=2, name=f"zb{t}_{jq}")
                for hh in range(2):
                    ouz = evicted[hh]
                    nc.sync.dma_start(
                        zb[8 * hh:8 * hh + 8, :],
                        ouz[64:65, :].rearrange("o (p q) -> o p q", p=8),
                    )
                rcp = znp.tile([16, 64], F32, tag="rcpb", bufs=2, name=f"rcp{t}_{jq}")
                nc.vector.reciprocal(rcp, zb)
                rcp16 = znp.tile([16, 64], BT16, tag="rcp16b", bufs=2, name=f"rcp16{t}_{jq}")
                nc.vector.tensor_copy(rcp16, rcp)
                for hh in range(2):
                    nc.sync.dma_start(
                        rcp_dram[jq, 2 * t + hh, :].rearrange("(p q) -> p q", p=8),
                        rcp16[8 * hh:8 * hh + 8, :],
                    )
                tmps = []
                for hh in range(2):
                    ouz = evicted[hh]
                    bc_sb = znp.tile([64, 512], BT16, tag="bc_sb", bufs=3, name=f"bs{t}_{jq}_{hh}")
                    src = rcp_dram[jq, 2 * t + hh, :]
                    bcast = bass_mod.AP(
                        tensor=src.tensor, offset=src.offset,
                        ap=[[0, 64]] + [list(a) for a in src.ap],
                    )
                    nc.sync.dma_start(bc_sb, bcast)
                    tmps.append((ouz, bc_sb))
                if not interleave_phase3:
                    for hh, (ouz, bc_sb) in enumerate(tmps):
                        if hh == 0:
                            nc.vector.tensor_mul(oT[t][0:64, qs2], ouz[0:64, :], bc_sb)
                        else:
                            tmp = znp.tile([64, 512], BT16, tag="tmp_o", bufs=2, name=f"tm{t}_{jq}")
                            nc.vector.tensor_mul(tmp, ouz[0:64, :], bc_sb)
                            nc.gpsimd.dma_start(oT[t][64:128, qs2], tmp)
                else:
                    # last pair of the last chunk: per-128-col muls, phase3
                    # m-chunk follows immediately after its slice is ready
                    for mi in range(4):
                        cs = slice(mi * 128, (mi + 1) * 128)
                        gs = slice(jq * 512 + mi * 128, jq * 512 + (mi + 1) * 128)
                        for hh, (ouz, bc_sb) in enumerate(tmps):
                            if hh == 0:
                                nc.vector.tensor_mul(oT[t][0:64, gs], ouz[0:64, cs], bc_sb[:, cs])
                            else:
                                tmp = znp.tile([64, 128], BT16, tag="tmp_os", bufs=4, name=f"tms{t}_{jq}_{mi}")
                                nc.vector.tensor_mul(tmp, ouz[0:64, cs], bc_sb[:, cs])
                                nc.sync.dma_start(oT[t][64:128, gs], tmp)
                        phase3_m(4 * jq + mi)

            phase1(0)
            pend = []          # (t, jq, evicted) not yet normalized
            for jq in range(NJQ):
                for t in range(NP):
                    ev = attention(t, jq)
                    if jq == 0 and t == 0 and NJQ > 1:
                        phase1(1)
                    if pend:
                        pt_, pjq_, pev_ = pend.pop(0)
                        normalize(pt_, pjq_, pev_)
                        if pt_ == NP - 1:
                            phase3(pjq_)
                    pend.append((t, jq, ev))
                if jq + 2 < NJQ:
                    phase1(jq + 2)
            # tail: all but the last pending entry normally; the last one
            # interleaves its normalization with phase3 m-chunks
            for pt_, pjq_, pev_ in pend[:-1]:
                normalize(pt_, pjq_, pev_)
                if pt_ == NP - 1:
                    phase3(pjq_)
            pt_, pjq_, pev_ = pend[-1]
            normalize(pt_, pjq_, pev_, interleave_phase3=True)

    nc.compile()
    return nc


def _host_prep(x, wq, bq, wk, bk, wv, wo):
    masks_np = np.zeros((128, 4, 512), dtype=BF16)
    qn = np.arange(512)[None, :]
    kn = np.arange(128)[:, None]
    for d in range(4):
        masks_np[:, d, :] = (qn >= kn + 128 * d).astype(BF16)

    per_g = []
    for g in range(G):
        cs = slice(g * HD, (g + 1) * HD)
        per_g.append({
            "wq": np.ascontiguousarray(wq[:, cs]).astype(BF16),
            "wk": np.ascontiguousarray(wk[:, cs]).astype(BF16),
            "wv": np.ascontiguousarray(wv[:, cs]).astype(BF16),
            "wo": np.ascontiguousarray(wo[cs, :]).astype(BF16),
            "bq": np.ascontiguousarray((bq[cs] / 8.0).reshape(NP, 128).T).astype(np.float32),
            "bk": np.ascontiguousarray(bk[cs].reshape(NP, 128).T).astype(np.float32),
            "masks": masks_np,
        })
    in_maps = []
    for c in range(8):
        b, g = divmod(c, G)
        m = dict(per_g[g])
        m["xT"] = np.ascontiguousarray(x[b].T).astype(BF16)
        in_maps.append(m)
    return in_maps


def kernel(x, wq, bq, wk, bk, wv, bv, wo, bo):
    x = np.asarray(x, dtype=np.float32)
    wq = np.asarray(wq, dtype=np.float32)
    bq = np.asarray(bq, dtype=np.float32)
    wk = np.asarray(wk, dtype=np.float32)
    bk = np.asarray(bk, dtype=np.float32)
    wv = np.asarray(wv, dtype=np.float32)
    bv = np.asarray(bv, dtype=np.float32)
    wo = np.asarray(wo, dtype=np.float32)
    bo = np.asarray(bo, dtype=np.float32)

    if "nc" not in _CACHED:
        _CACHED["nc"] = _build()
    nc = _CACHED["nc"]

    in_maps = _host_prep(x, wq, bq, wk, bk, wv, wo)
    res = run_bass_kernel_spmd(nc, in_maps, core_ids=list(range(8)))

    const_row = (bo.astype(np.float64) + bv.astype(np.float64) @ wo.astype(np.float64))
    out = np.empty((B, T, C), dtype=np.float32)
    for b in range(B):
        acc = res.results[2 * b]["y"].astype(np.float64)
        acc += res.results[2 * b + 1]["y"]
        acc += const_row[None, :]
        out[b] = acc.astype(np.float32)
    return out
